# revision 19
# baseline (speedup 1.0000x reference)
"""Trainium2 Bass kernel: 12-head self-attention block (B=2, N=4096, C=768).

Sharding: token-parallel over the 8192 (batch, token) rows. Core c (0..7)
handles batch c//4, query rows [(c%4)*1024, (c%4+1)*1024). Instead of
all-gathering K/V (4-rank ring AllGather measured ~60 GB/s -> ~360us of
dead time), every core redundantly computes K/V for its WHOLE batch
(~85us of extra warm PE time) - zero collectives, zero cross-core sync.

SPMD uniformity: all cores run the same graph; the host rotates each
core's token order so its own 1024 query tokens come first (attention is
permutation-invariant over keys, and k/v are derived in the same rotated
order, so results are exact).

Device pipeline per core (matmuls in float32r: full PE rate, ~tf32 precision):
  phase A (per 1024-token quarter): qkT[col,t] = Wqkv[:, :1536].T @ x^T
           (q rows only for quarter 0 -> SBUF; k rows -> DRAM scratch)
           v[t,col] = x @ Wqkv[:, 1536:] -> DRAM scratch as v_aug[t,h,65]
           with a ones column (AV matmul then yields softmax denominators
           for free as output row 64).
  phase C: per head-pair hp, query-block qb (512), key-chunk group (3x128):
           scoresT[key,q] = kT_h.T @ qT_h      (K=64, heads at partition
           bases 0/64)
           eT = exp(SCALE * scoresT)           (ACT reads 3-bank PSUM tile)
           av[0:65] += v_aug.T @ eT            (row 64 = denominators)
           denominators -> gpsimd partition_broadcast -> DVE divide
  phase D: out[t,c] = tokensT.T @ Wproj + bproj (bias via broadcast + add)
"""

import sys

import numpy as np

try:
    import concourse  # noqa: F401
except ImportError:  # pragma: no cover
    sys.path.insert(0, "/opt/trn_rl_repo")

import concourse.bass as bass  # noqa: F401
import concourse.mybir as mybir
import concourse.tile as tile
from concourse import bacc
from concourse.bass_utils import run_bass_kernel_spmd

B, N, C = 2, 4096, 768
H, D = 12, 64
NT = 1024  # query tokens per core
SCALE = float(D) ** -0.5
NCORES = 8
KC = N // 128  # 32 key chunks per batch
VW = D + 1  # v_aug row width per head: [v(64), ones]

F32 = mybir.dt.float32
F32R = mybir.dt.float32r
EXP = mybir.ActivationFunctionType.Exp
DIV = mybir.AluOpType.divide
MUL = mybir.AluOpType.mult
ADD = mybir.AluOpType.add

USE_DIVIDE = True  # DVE divide vs reciprocal+mult for the softmax denom


def build_graph():
    nc = bacc.Bacc(
        "TRN2", target_bir_lowering=False, debug=False, num_devices=NCORES
    )

    xT_e = nc.declare_dram_parameter("xT", [C, N], F32R, isOutput=False)
    wqkv_e = nc.declare_dram_parameter("Wqkv", [C, 3 * C], F32R, isOutput=False)
    wproj_e = nc.declare_dram_parameter("Wproj", [C, C], F32R, isOutput=False)
    bproj_e = nc.declare_dram_parameter("bproj", [1, C], F32R, isOutput=False)
    ones_e = nc.declare_dram_parameter("ones", [128, 128], F32R, isOutput=False)
    out_e = nc.declare_dram_parameter("out", [NT, C], F32, isOutput=True)

    with tile.TileContext(nc) as tc:
        _build_body(nc, tc, xT_e, wqkv_e, wproj_e, bproj_e, ones_e, out_e)
    nc.finalize()
    return nc


def _build_body(nc, tc, xT_e, wqkv_e, wproj_e, bproj_e, ones_e, out_e):
    with (
        tc.tile_pool(name="dram", bufs=1, space="DRAM") as dram,
        tc.tile_pool(name="persist", bufs=1) as persist,
    ):
        # ---- persistent SBUF ----
        qT_sb = persist.tile([128, 6, NT], F32R, tag="qT")
        tokT = persist.tile([128, 6, NT], F32R, tag="tokT")
        ones_sb = persist.tile([128, 128], F32R, tag="ones")
        nc.sync.dma_start(ones_sb[:], ones_e[:])

        # ---- DRAM scratch (local, no collectives) ----
        scr_k = dram.tile([C, N], F32R, tag="sk")
        scr_v = dram.tile([N, H, VW], F32R, tag="sv")

        # ================= phase A: qkv projection =================
        with (
            tc.tile_pool(name="pa_w", bufs=1) as paw,
            tc.tile_pool(name="pa_x", bufs=2) as pax,
            tc.tile_pool(name="pa_st", bufs=4) as past,
            tc.tile_pool(name="pa_psum", bufs=4, space="PSUM") as pap,
        ):
            wqkv_sb = paw.tile([128, 6, 3 * C], F32R, tag="wqkv")
            for kc in range(6):
                nc.sync.dma_start(
                    wqkv_sb[:, kc, :], wqkv_e[kc * 128 : (kc + 1) * 128, :]
                )

            for tq in range(4):  # 1024-token quarters
                tq0 = tq * NT
                xq = pax.tile([128, 6, NT], F32R, tag="xq")
                for kc in range(6):
                    nc.sync.dma_start(
                        xq[:, kc, :],
                        xT_e[kc * 128 : (kc + 1) * 128, tq0 : tq0 + NT],
                    )

                # q (quarter 0 only) + k columns, transposed orientation
                ccs = range(12) if tq == 0 else range(6, 12)
                for cc in ccs:
                    for th in range(2):
                        pj = pap.tile([128, 512], F32, tag="pj")
                        for kc in range(6):
                            nc.tensor.matmul(
                                pj[:],
                                wqkv_sb[:, kc, cc * 128 : (cc + 1) * 128],
                                xq[:, kc, th * 512 : (th + 1) * 512],
                                start=(kc == 0),
                                stop=(kc == 5),
                            )
                        if cc < 6:
                            nc.vector.tensor_copy(
                                qT_sb[:, cc, th * 512 : (th + 1) * 512], pj[:]
                            )
                        else:
                            kst = past.tile([128, 512], F32R, tag="kst")
                            nc.vector.tensor_copy(kst[:], pj[:])
                            nc.sync.dma_start(
                                scr_k[
                                    (cc - 6) * 128 : (cc - 5) * 128,
                                    tq0 + th * 512 : tq0 + (th + 1) * 512,
                                ],
                                kst[:],
                            )

                # v columns (token-major)
                for tcn in range(8):
                    trow = tq0 + tcn * 128
                    for vh in range(2):
                        pj = pap.tile([128, 384], F32, tag="pjv")
                        for kc in range(6):
                            nc.tensor.matmul(
                                pj[:],
                                xq[:, kc, tcn * 128 : (tcn + 1) * 128],
                                wqkv_sb[
                                    :, kc, 2 * C + vh * 384 : 2 * C + (vh + 1) * 384
                                ],
                                start=(kc == 0),
                                stop=(kc == 5),
                            )
                        vst = past.tile([128, 384], F32R, tag="vst")
                        nc.vector.tensor_copy(vst[:], pj[:])
                        nc.sync.dma_start(
                            scr_v[trow : trow + 128, vh * 6 : (vh + 1) * 6, 0:D],
                            vst[:].rearrange("p (h d) -> p h d", d=D),
                        )
                    nc.sync.dma_start(
                        scr_v[trow : trow + 128, :, D : D + 1],
                        ones_sb[:, 0:H].rearrange("p (h o) -> p h o", o=1),
                    )

        # ================= phase C: attention =================
        groups = [3] * 10 + [2]
        with (
            tc.tile_pool(name="kt_pool", bufs=3) as ktp,
            tc.tile_pool(name="vt_pool", bufs=3) as vtp,
            tc.tile_pool(name="et_pool", bufs=4) as etp,
            tc.tile_pool(name="sc_pool", bufs=2, space="PSUM") as scp,
            tc.tile_pool(name="av_pool", bufs=2, space="PSUM") as avp,
            tc.tile_pool(name="small", bufs=4) as smp,
        ):
            for hp in range(6):
                kt = ktp.tile([128, N], F32R, tag="kt")
                nc.sync.dma_start(kt[:], scr_k[hp * 128 : (hp + 1) * 128, :])
                vt = vtp.tile([128, KC, 2, VW], F32R, tag="vt")
                nc.sync.dma_start(
                    vt[:],
                    scr_v[:].rearrange("(c p) h w -> p c h w", p=128)[
                        :, :, 2 * hp : 2 * hp + 2, :
                    ],
                )
                for qb in range(2):
                    qsl = slice(qb * 512, (qb + 1) * 512)
                    av0 = avp.tile([128, 512], F32, tag="av")
                    av1 = avp.tile([128, 512], F32, tag="av")
                    kc0 = 0
                    for gs in groups:
                        sc0 = scp.tile([128, 1536], F32, tag="sc")
                        sc1 = scp.tile([128, 1536], F32, tag="sc")
                        for j in range(gs):
                            kc = kc0 + j
                            ksl = slice(kc * 128, (kc + 1) * 128)
                            jsl = slice(j * 512, (j + 1) * 512)
                            nc.tensor.matmul(
                                sc0[:, jsl],
                                kt[0:64, ksl],
                                qT_sb[0:64, hp, qsl],
                                start=True,
                                stop=True,
                            )
                            nc.tensor.matmul(
                                sc1[:, jsl],
                                kt[64:128, ksl],
                                qT_sb[64:128, hp, qsl],
                                start=True,
                                stop=True,
                            )
                        e0 = etp.tile([128, 1536], F32R, tag="et")
                        e1 = etp.tile([128, 1536], F32R, tag="et")
                        gsl = slice(0, gs * 512)
                        nc.scalar.activation(e0[:, gsl], sc0[:, gsl], EXP, scale=SCALE)
                        nc.scalar.activation(e1[:, gsl], sc1[:, gsl], EXP, scale=SCALE)
                        for j in range(gs):
                            kc = kc0 + j
                            jsl = slice(j * 512, (j + 1) * 512)
                            nc.tensor.matmul(
                                av0[0:65, :],
                                vt[:, kc, 0, :],  # [v(64) | ones]
                                e0[:, jsl],
                                start=(kc == 0),
                                stop=(kc == KC - 1),
                            )
                            nc.tensor.matmul(
                                av1[0:65, :],
                                vt[:, kc, 1, :],
                                e1[:, jsl],
                                start=(kc == 0),
                                stop=(kc == KC - 1),
                            )
                        kc0 += gs

                    for hd, av in ((0, av0), (1, av1)):
                        # one copy releases the PSUM bank; the recip chain
                        # then runs off the PE critical path
                        av_sb = smp.tile([65, 512], F32, tag="av_sb")
                        nc.vector.tensor_copy(av_sb[:], av[0:65, :])
                        rec = smp.tile([1, 512], F32, tag="rec")
                        nc.vector.reciprocal(rec[:], av_sb[64:65, :])
                        bc = smp.tile([64, 512], F32, tag="bc")
                        nc.gpsimd.partition_broadcast(bc[:], rec[:])
                        if hd == 0:
                            nc.vector.tensor_tensor(
                                out=tokT[0:64, hp, qsl],
                                in0=av_sb[0:64, :],
                                in1=bc[:],
                                op=MUL,
                            )
                        else:
                            tmp = smp.tile([64, 512], F32R, tag="tmp")
                            nc.vector.tensor_tensor(
                                out=tmp[:], in0=av_sb[0:64, :], in1=bc[:], op=MUL
                            )
                            # partition-shifting copy (base 0 -> 64) via DMA
                            nc.sync.dma_start(tokT[64:128, hp, qsl], tmp[:])

        # ================= phase D: output projection =================
        with (
            tc.tile_pool(name="pd_w", bufs=1) as pdw,
            tc.tile_pool(name="pd_psum", bufs=4, space="PSUM") as pdp,
            tc.tile_pool(name="pd_sbuf", bufs=4) as pds,
        ):
            wproj_sb = pdw.tile([128, 6, C], F32R, tag="wproj")
            bproj_sb = pdw.tile([1, C], F32R, tag="bproj")
            bproj_bc = pdw.tile([128, C], F32R, tag="bproj_bc")
            nc.sync.dma_start(bproj_sb[:], bproj_e[:])
            nc.gpsimd.partition_broadcast(bproj_bc[:], bproj_sb[:])
            for cc in range(6):
                nc.sync.dma_start(
                    wproj_sb[:, cc, :], wproj_e[cc * 128 : (cc + 1) * 128, :]
                )
            for tcn in range(8):
                for vh in range(2):
                    csl = slice(vh * 384, (vh + 1) * 384)
                    pj = pdp.tile([128, 384], F32, tag="pd")
                    for cc in range(6):
                        nc.tensor.matmul(
                            pj[:],
                            tokT[:, cc, tcn * 128 : (tcn + 1) * 128],
                            wproj_sb[:, cc, csl],
                            start=(cc == 0),
                            stop=(cc == 5),
                        )
                    ot = pds.tile([128, 384], F32, tag="ot")
                    nc.vector.tensor_tensor(
                        out=ot[:],
                        in0=pj[:],
                        in1=bproj_bc[:, csl].bitcast(F32),
                        op=ADD,
                    )
                    nc.sync.dma_start(
                        out_e[tcn * 128 : (tcn + 1) * 128, csl], ot[:]
                    )


_CACHE = {}


def _get_graph():
    if "nc" not in _CACHE:
        _CACHE["nc"] = build_graph()
    return _CACHE["nc"]


def make_in_maps(x, W_qkv, W_proj, b_proj):
    x = np.asarray(x, dtype=np.float32)
    W_qkv = np.ascontiguousarray(np.asarray(W_qkv, dtype=np.float32))
    W_proj = np.ascontiguousarray(np.asarray(W_proj, dtype=np.float32))
    b_proj = np.asarray(b_proj, dtype=np.float32).reshape(1, C)
    ones = np.ones((128, 128), dtype=np.float32)
    in_maps = []
    for c in range(NCORES):
        bb, r0 = c // 4, (c % 4) * NT
        idx = np.r_[r0 : r0 + NT, 0:r0, r0 + NT : N]
        xT = np.ascontiguousarray(x[bb][idx].T)  # own tokens first
        in_maps.append(
            {
                "xT": xT,
                "Wqkv": W_qkv,
                "Wproj": W_proj,
                "bproj": b_proj,
                "ones": ones,
            }
        )
    return in_maps


def run(x, W_qkv, W_proj, b_proj, trace=False):
    nc = _get_graph()
    in_maps = make_in_maps(x, W_qkv, W_proj, b_proj)
    res = run_bass_kernel_spmd(
        nc, in_maps, core_ids=list(range(NCORES)), trace=trace
    )
    out = np.zeros((B, N, C), dtype=np.float32)
    for c in range(NCORES):
        bb, r0 = c // 4, (c % 4) * NT
        out[bb, r0 : r0 + NT, :] = res.results[c]["out"]
    return out, res


def kernel(x, W_qkv, W_proj, b_proj):
    out, _ = run(x, W_qkv, W_proj, b_proj, trace=False)
    return out
